# revision 1
# baseline (speedup 1.0000x reference)
"""Trainium2 Bass kernel for DenseConv2d.

Conv2d: input (32,128,56,56) f32, weight (256,128,3,3) f32, bias (256,) f32,
stride 1, pad 1, dilation 1 -> output (32,256,56,56) f32.

Strategy: data-parallel over batch across 8 NeuronCores (4 images per core).
Per core the conv is computed as 9 accumulated matmuls (one per kernel tap)
into PSUM: out[co, pix] += W[kh,kw][ci,co].T @ x_pad[ci, shifted pix window].
Operands stream through the PE array as float32r (~1.1 cycles/row sustained).
Input is chunked (2 row-blocks + halo per DMA) on the scalar-engine HWDGE
queue so the first matmul starts as early as possible; output DMAs ride the
sync queue. A few warmup matmuls on scratch data run during the input DMA
wait to lift the PE HAM clock-gate to 2.4 GHz before real work arrives.
Layout prep (padding, channel-major transpose) is host-side numpy.
"""

import sys

if "/opt/trn_rl_repo" not in sys.path:
    sys.path.insert(0, "/opt/trn_rl_repo")

import numpy as np

N_CORES = 8
N, CI, H, W = 32, 128, 56, 56
CO, KH, KW = 256, 3, 3
NP_CORE = N // N_CORES          # images per core
HP, WP = H + 2, W + 2           # padded spatial dims
COT = CO // 128                 # out-channel tiles of 128
RB = 8                          # output rows per matmul block
NBLK = H // RB                  # row blocks per image
CHROWS = 2 * RB + 2             # input rows per chunk (2 blocks + halo)
NCH = 4                         # chunks per image (last one is short)
N_WARMUP = 5                    # PE warmup matmuls

_CACHE = {}


def _build_program():
    import concourse.mybir as mybir
    from concourse import bacc
    from concourse.tile import TileContext

    nc = bacc.Bacc(None, target_bir_lowering=False)

    x_d = nc.dram_tensor("x", [CI, NP_CORE, HP, WP], mybir.dt.float32r,
                         kind="ExternalInput")
    w_d = nc.dram_tensor("w", [CI, COT, KH * KW, 128], mybir.dt.float32r,
                         kind="ExternalInput")
    b_d = nc.dram_tensor("b2", [128, COT], mybir.dt.float32,
                         kind="ExternalInput")
    y_d = nc.dram_tensor("y", [COT, 128, NP_CORE, H, W], mybir.dt.float32,
                         kind="ExternalOutput")

    f32 = mybir.dt.float32
    f32r = mybir.dt.float32r

    with TileContext(nc) as tc:
        with (
            tc.tile_pool(name="xin", bufs=1) as xpool,
            tc.tile_pool(name="wpool", bufs=1) as wpool,
            tc.tile_pool(name="bpool", bufs=1) as bpool,
            tc.tile_pool(name="psum", bufs=8, space="PSUM") as ppool,
            tc.tile_pool(name="out", bufs=6) as opool,
        ):
            # PE warmup on scratch data, concurrent with the input DMAs,
            # so the HAM clock-gate is at 2.4 GHz when real matmuls start.
            scratch = xpool.tile([CI, RB * W], mybir.dt.bfloat16,
                                 tag="scratch")
            nc.vector.memset(scratch, 0.0)
            wups = ppool.tile([128, RB * W], f32, tag="ps")
            for _ in range(N_WARMUP):
                nc.tensor.matmul(wups, scratch[:, 0:128], scratch,
                                 start=True, stop=True)
            # Tiny-warmup tail (~53 ns each, ~1.6 us total): bridges the
            # PE-busy window from the big warmups to the first input
            # chunk's arrival, so HAM is at 2.4 GHz and the real stream
            # never starts cold.
            for _ in range(30):
                nc.tensor.matmul(wups[:, 0:64], scratch[:, 0:128],
                                 scratch[:, 0:64], start=True, stop=True)

            # Weights split by out-channel tile so the first matmul group
            # only waits for w[cot=0] + the first input chunk (~0.7 MB).
            wt = []
            for cot in range(COT):
                wtile = wpool.tile([CI, KH * KW, 128], f32r, tag=f"w{cot}")
                wt.append(wtile)
            bt = bpool.tile([128, COT], f32)

            def wslice(pos, cot):
                return wt[cot][:, pos, :]

            # Input chunks per image: (padded_row0, n_blocks). The first is
            # a single block so the very first matmul group's data arrives
            # fast; block b lives in chunk CHMAP[b] at local row CHLOC[b].
            CHUNKS = [(0, 1), (RB, 2), (3 * RB, 2), (5 * RB, 2)]
            CHMAP, CHLOC = {}, {}
            b = 0
            for ci_, (r0_, nb_) in enumerate(CHUNKS):
                for j in range(nb_):
                    CHMAP[b], CHLOC[b] = ci_, j * RB
                    b += 1
            xt = {}

            def x_chunk_dma(img, ch, eng):
                r0, nb = CHUNKS[ch]
                rows = min(nb * RB + 2, HP - r0)
                t = xpool.tile([CI, rows, WP], f32r, tag=f"x{img}_{ch}")
                eng.dma_start(out=t, in_=x_d[:, img, r0:r0 + rows, :])
                xt[img, ch] = t

            # Critical path: the first matmul group needs x0 chunk0 plus all
            # 9 taps of w[cot0]; spread those over both HWDGE queues.
            nc.scalar.dma_start(out=wt[0][:, 0:5, :], in_=w_d[:, 0, 0:5, :])
            x_chunk_dma(0, 0, nc.sync)
            nc.sync.dma_start(out=wt[0][:, 5:9, :], in_=w_d[:, 0, 5:9, :])
            x_chunk_dma(0, 1, nc.scalar)
            nc.sync.dma_start(out=wt[1], in_=w_d[:, 1, :, :])
            x_chunk_dma(0, 2, nc.scalar)
            x_chunk_dma(0, 3, nc.sync)
            nc.scalar.dma_start(out=bt, in_=b_d[:, :])
            for img in range(1, NP_CORE):
                for ch in range(len(CHUNKS)):
                    x_chunk_dma(img, ch, nc.scalar)

            for img in range(NP_CORE):
                for cot in range(COT):
                    for blk in range(NBLK):
                        ps = ppool.tile([128, RB, W], f32, tag="ps")
                        ch, r0 = CHMAP[blk], CHLOC[blk]
                        for pos in range(KH * KW):
                            kh, kw = divmod(pos, KW)
                            rhs = xt[img, ch][:, r0 + kh:r0 + kh + RB,
                                              kw:kw + W]
                            nc.tensor.matmul(
                                ps, wslice(pos, cot), rhs,
                                start=(pos == 0), stop=(pos == KH * KW - 1),
                            )
                        last = (img == NP_CORE - 1 and cot == COT - 1
                                and blk == NBLK - 1)
                        if last:
                            # Tail: ship the final block as two half-copies on
                            # both queues so the store pipeline drains sooner.
                            h = RB // 2
                            ot1 = opool.tile([128, h, W], f32, tag="ot1")
                            nc.vector.tensor_scalar_add(
                                ot1, ps[:, 0:h, :], bt[:, cot:cot + 1])
                            nc.sync.dma_start(
                                out=y_d[cot, :, img,
                                        blk * RB:blk * RB + h, :], in_=ot1)
                            ot2 = opool.tile([128, h, W], f32, tag="ot2")
                            nc.vector.tensor_scalar_add(
                                ot2, ps[:, h:RB, :], bt[:, cot:cot + 1])
                            nc.scalar.dma_start(
                                out=y_d[cot, :, img,
                                        blk * RB + h:blk * RB + RB, :],
                                in_=ot2)
                        else:
                            ot = opool.tile([128, RB, W], f32)
                            nc.vector.tensor_scalar_add(
                                ot, ps, bt[:, cot:cot + 1])
                            nc.sync.dma_start(
                                out=y_d[cot, :, img,
                                        blk * RB:blk * RB + RB, :], in_=ot)

    nc.compile()
    return nc


def prep_in_maps(input, weight, bias):
    """Host-side layout prep -> one in_map per core."""
    xp = np.pad(input, ((0, 0), (0, 0), (1, 1), (1, 1)))
    # weight [co, ci, kh, kw] -> [ci, cot, (kh kw), cop]
    wr = np.ascontiguousarray(
        weight.transpose(1, 2, 3, 0).reshape(CI, KH * KW, COT, 128)
        .transpose(0, 2, 1, 3))
    b2 = np.ascontiguousarray(bias.reshape(COT, 128).T)

    in_maps = []
    for c in range(N_CORES):
        xc = np.ascontiguousarray(
            xp[c * NP_CORE:(c + 1) * NP_CORE].transpose(1, 0, 2, 3))
        in_maps.append({"x": xc, "w": wr, "b2": b2})
    return in_maps


def kernel(input, weight, bias):
    input = np.asarray(input, dtype=np.float32)
    weight = np.asarray(weight, dtype=np.float32)
    bias = np.asarray(bias, dtype=np.float32)

    if "nc" not in _CACHE:
        _CACHE["nc"] = _build_program()
    nc = _CACHE["nc"]

    from concourse.bass_utils import run_bass_kernel_spmd

    in_maps = prep_in_maps(input, weight, bias)
    res = run_bass_kernel_spmd(nc, in_maps, core_ids=list(range(N_CORES)))

    out = np.empty((N, CO, H, W), dtype=np.float32)
    for c in range(N_CORES):
        y = res.results[c]["y"]  # [COT, 128, NP_CORE, H, W]
        out[c * NP_CORE:(c + 1) * NP_CORE] = (
            y.transpose(2, 0, 1, 3, 4).reshape(NP_CORE, CO, H, W))
    return out



# revision 2
# speedup vs baseline: 1.0760x; 1.0760x over previous
"""Trainium2 Bass kernel for DenseConv2d.

Conv2d: input (32,128,56,56) f32, weight (256,128,3,3) f32, bias (256,) f32,
stride 1, pad 1, dilation 1 -> output (32,256,56,56) f32.

Strategy: data-parallel over batch across 8 NeuronCores (4 images per core).
Per core the conv is computed as 9 accumulated matmuls (one per kernel tap)
into PSUM: out[co, pix] += W[kh,kw][ci,co].T @ x_pad[ci, shifted pix window].
Operands are bf16 (PSUM accumulation stays fp32): streaming rate matches
f32r (1 col/cycle) but input DMA bytes halve and LDWEIGHTS gets the
fast-weight-load path.  Loop order is img -> block -> cot so each input
chunk feeds 18 back-to-back matmuls as soon as it lands.  Output stores
alternate between the sync and scalar HWDGE queues so the store stream
(12.8 MB/core fp32) never backs up behind a single queue.  A few warmup
matmuls on scratch data run during the input DMA wait to lift the PE HAM
clock-gate to 2.4 GHz before real work arrives.  Layout prep (padding,
channel-major transpose, bf16 cast) is host-side numpy.
"""

import sys

if "/opt/trn_rl_repo" not in sys.path:
    sys.path.insert(0, "/opt/trn_rl_repo")

import numpy as np

N_CORES = 8
N, CI, H, W = 32, 128, 56, 56
CO, KH, KW = 256, 3, 3
NP_CORE = N // N_CORES          # images per core
HP, WP = H + 2, W + 2           # padded spatial dims
COT = CO // 128                 # out-channel tiles of 128
RB = 8                          # output rows per matmul block
NBLK = H // RB                  # row blocks per image
NCH = 4                         # chunks per image (last one is short)
N_WARMUP = 5                    # big PE warmup matmuls
N_TINY = 8                      # short bridging warmups

_CACHE = {}


def _build_program():
    import concourse.mybir as mybir
    from concourse import bacc
    from concourse.tile import TileContext

    nc = bacc.Bacc(None, target_bir_lowering=False)

    bf16 = mybir.dt.bfloat16
    f32 = mybir.dt.float32

    x_d = nc.dram_tensor("x", [CI, NP_CORE, HP, WP], bf16,
                         kind="ExternalInput")
    w_d = nc.dram_tensor("w", [CI, COT, KH * KW, 128], bf16,
                         kind="ExternalInput")
    b_d = nc.dram_tensor("b2", [128, COT], f32,
                         kind="ExternalInput")
    y_d = nc.dram_tensor("y", [COT, 128, NP_CORE, H, W], f32,
                         kind="ExternalOutput")

    with TileContext(nc) as tc:
        with (
            tc.tile_pool(name="xin", bufs=1) as xpool,
            tc.tile_pool(name="wpool", bufs=1) as wpool,
            tc.tile_pool(name="bpool", bufs=1) as bpool,
            tc.tile_pool(name="psum", bufs=8, space="PSUM") as ppool,
            tc.tile_pool(name="out", bufs=6) as opool,
        ):
            # PE warmup on scratch data, concurrent with the input DMAs,
            # so the HAM clock-gate is at 2.4 GHz when real matmuls start.
            scratch = xpool.tile([CI, RB * W], bf16, tag="scratch")
            nc.vector.memset(scratch, 0.0)
            wups = ppool.tile([128, RB * W], f32, tag="ps")
            for _ in range(N_WARMUP):
                nc.tensor.matmul(wups, scratch[:, 0:128], scratch,
                                 start=True, stop=True)
            for _ in range(N_TINY):
                nc.tensor.matmul(wups[:, 0:64], scratch[:, 0:128],
                                 scratch[:, 0:64], start=True, stop=True)

            # Weights split by out-channel tile so the first matmul group
            # only waits for w[cot=0] + the first input chunk.
            wt = []
            for cot in range(COT):
                wtile = wpool.tile([CI, KH * KW, 128], bf16, tag=f"w{cot}")
                wt.append(wtile)
            bt = bpool.tile([128, COT], f32)

            def wslice(pos, cot):
                return wt[cot][:, pos, :]

            # Input chunks per image: (padded_row0, n_blocks). The first is
            # a single block so the very first matmul group's data arrives
            # fast; block b lives in chunk CHMAP[b] at local row CHLOC[b].
            CHUNKS = [(0, 1), (RB, 2), (3 * RB, 2), (5 * RB, 2)]
            CHMAP, CHLOC = {}, {}
            b = 0
            for ci_, (r0_, nb_) in enumerate(CHUNKS):
                for j in range(nb_):
                    CHMAP[b], CHLOC[b] = ci_, j * RB
                    b += 1
            xt = {}

            def x_chunk_dma(img, ch, eng):
                r0, nb = CHUNKS[ch]
                rows = min(nb * RB + 2, HP - r0)
                t = xpool.tile([CI, rows, WP], bf16, tag=f"x{img}_{ch}")
                eng.dma_start(out=t, in_=x_d[:, img, r0:r0 + rows, :])
                xt[img, ch] = t

            # Critical path: the first matmul group needs x0 chunk0 plus all
            # 9 taps of w[cot0]; spread those over both HWDGE queues.  All
            # input DMAs are issued up front so they sit ahead of every
            # output store in each queue's ring.
            nc.scalar.dma_start(out=wt[0][:, 0:5, :], in_=w_d[:, 0, 0:5, :])
            x_chunk_dma(0, 0, nc.sync)
            nc.sync.dma_start(out=wt[0][:, 5:9, :], in_=w_d[:, 0, 5:9, :])
            x_chunk_dma(0, 1, nc.scalar)
            nc.sync.dma_start(out=wt[1], in_=w_d[:, 1, :, :])
            x_chunk_dma(0, 2, nc.scalar)
            x_chunk_dma(0, 3, nc.sync)
            nc.scalar.dma_start(out=bt, in_=b_d[:, :])
            for img in range(1, NP_CORE):
                for ch in range(len(CHUNKS)):
                    x_chunk_dma(img, ch, nc.scalar if ch % 2 else nc.sync)

            out_parity = 0
            for img in range(NP_CORE):
                for blk in range(NBLK):
                    for cot in range(COT):
                        ps = ppool.tile([128, RB, W], f32, tag="ps")
                        ch, r0 = CHMAP[blk], CHLOC[blk]
                        for pos in range(KH * KW):
                            kh, kw = divmod(pos, KW)
                            rhs = xt[img, ch][:, r0 + kh:r0 + kh + RB,
                                              kw:kw + W]
                            nc.tensor.matmul(
                                ps, wslice(pos, cot), rhs,
                                start=(pos == 0), stop=(pos == KH * KW - 1),
                            )
                        last = (img == NP_CORE - 1 and cot == COT - 1
                                and blk == NBLK - 1)
                        if last:
                            # Tail: bias-add + store in half-blocks on both
                            # queues so the drain pipeline is short.
                            h = RB // 2
                            ot1 = opool.tile([128, h, W], f32, tag="ot1")
                            nc.vector.tensor_scalar_add(
                                ot1, ps[:, 0:h, :], bt[:, cot:cot + 1])
                            nc.sync.dma_start(
                                out=y_d[cot, :, img,
                                        blk * RB:blk * RB + h, :], in_=ot1)
                            ot2 = opool.tile([128, h, W], f32, tag="ot2")
                            nc.vector.tensor_scalar_add(
                                ot2, ps[:, h:RB, :], bt[:, cot:cot + 1])
                            nc.scalar.dma_start(
                                out=y_d[cot, :, img,
                                        blk * RB + h:blk * RB + RB, :],
                                in_=ot2)
                        else:
                            ot = opool.tile([128, RB, W], f32)
                            nc.vector.tensor_scalar_add(
                                ot, ps, bt[:, cot:cot + 1])
                            eng = nc.sync if out_parity == 0 else nc.scalar
                            out_parity ^= 1
                            eng.dma_start(
                                out=y_d[cot, :, img,
                                        blk * RB:blk * RB + RB, :], in_=ot)

    nc.compile()
    return nc


def prep_in_maps(input, weight, bias):
    """Host-side layout prep -> one in_map per core."""
    import ml_dtypes

    bf16 = ml_dtypes.bfloat16
    xp = np.pad(input, ((0, 0), (0, 0), (1, 1), (1, 1))).astype(bf16)
    # weight [co, ci, kh, kw] -> [ci, cot, (kh kw), cop]
    wr = np.ascontiguousarray(
        weight.transpose(1, 2, 3, 0).reshape(CI, KH * KW, COT, 128)
        .transpose(0, 2, 1, 3)).astype(bf16)
    b2 = np.ascontiguousarray(bias.reshape(COT, 128).T)

    in_maps = []
    for c in range(N_CORES):
        xc = np.ascontiguousarray(
            xp[c * NP_CORE:(c + 1) * NP_CORE].transpose(1, 0, 2, 3))
        in_maps.append({"x": xc, "w": wr, "b2": b2})
    return in_maps


def kernel(input, weight, bias):
    input = np.asarray(input, dtype=np.float32)
    weight = np.asarray(weight, dtype=np.float32)
    bias = np.asarray(bias, dtype=np.float32)

    if "nc" not in _CACHE:
        _CACHE["nc"] = _build_program()
    nc = _CACHE["nc"]

    from concourse.bass_utils import run_bass_kernel_spmd

    in_maps = prep_in_maps(input, weight, bias)
    res = run_bass_kernel_spmd(nc, in_maps, core_ids=list(range(N_CORES)))

    out = np.empty((N, CO, H, W), dtype=np.float32)
    for c in range(N_CORES):
        y = res.results[c]["y"]  # [COT, 128, NP_CORE, H, W]
        out[c * NP_CORE:(c + 1) * NP_CORE] = (
            y.transpose(2, 0, 1, 3, 4).reshape(NP_CORE, CO, H, W))
    return out


# revision 5
# speedup vs baseline: 1.1074x; 1.0292x over previous
"""Trainium2 Bass kernel for DenseConv2d.

Conv2d: input (32,128,56,56) f32, weight (256,128,3,3) f32, bias (256,) f32,
stride 1, pad 1, dilation 1 -> output (32,256,56,56) f32.

Strategy: data-parallel over batch across 8 NeuronCores (4 images per core).
Per core the conv is computed as 9 accumulated matmuls (one per kernel tap)
into PSUM: out[co, pix] += W[kh,kw][ci,co].T @ x_pad[ci, shifted pix window].
Operands are bf16 (PSUM accumulation stays fp32): streaming rate matches
f32r (1 col/cycle) but input DMA bytes halve and LDWEIGHTS gets the
fast-weight-load path.  Loop order is img -> block -> cot so each input
chunk feeds 18 back-to-back matmuls as soon as it lands.  Output stores
alternate between the sync and scalar HWDGE queues so the store stream
(12.8 MB/core fp32) never backs up behind a single queue.  A few warmup
matmuls on scratch data run during the input DMA wait to lift the PE HAM
clock-gate to 2.4 GHz before real work arrives.  Layout prep (padding,
channel-major transpose, bf16 cast) is host-side numpy.
"""

import sys

if "/opt/trn_rl_repo" not in sys.path:
    sys.path.insert(0, "/opt/trn_rl_repo")

import numpy as np

N_CORES = 8
N, CI, H, W = 32, 128, 56, 56
CO, KH, KW = 256, 3, 3
NP_CORE = N // N_CORES          # images per core
HP, WP = H + 2, W + 2           # padded spatial dims
COT = CO // 128                 # out-channel tiles of 128
RB = 8                          # output rows per matmul block
NBLK = H // RB                  # row blocks per image
NCH = 4                         # chunks per image (last one is short)
N_WARMUP = 5                    # big PE warmup matmuls
N_TINY = 4                      # short bridging warmups

_CACHE = {}


def _build_program():
    import concourse.mybir as mybir
    from concourse import bacc
    from concourse.tile import TileContext

    nc = bacc.Bacc(None, target_bir_lowering=False)

    bf16 = mybir.dt.bfloat16
    f32 = mybir.dt.float32

    x_d = nc.dram_tensor("x", [CI, NP_CORE, HP, WP], bf16,
                         kind="ExternalInput")
    w_d = nc.dram_tensor("w", [CI, COT, KH * KW, 128], bf16,
                         kind="ExternalInput")
    b_d = nc.dram_tensor("b2", [128, COT], f32,
                         kind="ExternalInput")
    y_d = nc.dram_tensor("y", [COT, 128, NP_CORE, H, W], f32,
                         kind="ExternalOutput")

    with TileContext(nc) as tc:
        with (
            tc.tile_pool(name="xin", bufs=1) as xpool,
            tc.tile_pool(name="wpool", bufs=1) as wpool,
            tc.tile_pool(name="bpool", bufs=1) as bpool,
            tc.tile_pool(name="psum", bufs=8, space="PSUM") as ppool,
            tc.tile_pool(name="out", bufs=6) as opool,
        ):
            # PE warmup on scratch data, concurrent with the input DMAs,
            # so the HAM clock-gate is at 2.4 GHz when real matmuls start.
            scratch = xpool.tile([CI, RB * W], bf16, tag="scratch")
            nc.vector.memset(scratch, 0.0)
            wups = ppool.tile([128, RB * W], f32, tag="ps")
            for _ in range(N_WARMUP):
                nc.tensor.matmul(wups, scratch[:, 0:128], scratch,
                                 start=True, stop=True)
            for _ in range(N_TINY):
                nc.tensor.matmul(wups[:, 0:64], scratch[:, 0:128],
                                 scratch[:, 0:64], start=True, stop=True)

            # Weights split by out-channel tile so the first matmul group
            # only waits for w[cot=0] + the first input chunk.
            wt = []
            for cot in range(COT):
                wtile = wpool.tile([CI, KH * KW, 128], bf16, tag=f"w{cot}")
                wt.append(wtile)
            bt = bpool.tile([128, COT], f32)

            def wslice(pos, cot):
                return wt[cot][:, pos, :]

            # Input chunks per image: (padded_row0, n_blocks). The first is
            # a single block so the very first matmul group's data arrives
            # fast; block b lives in chunk CHMAP[b] at local row CHLOC[b].
            CHUNKS = [(0, 1), (RB, 2), (3 * RB, 2), (5 * RB, 2)]
            CHMAP, CHLOC = {}, {}
            b = 0
            for ci_, (r0_, nb_) in enumerate(CHUNKS):
                for j in range(nb_):
                    CHMAP[b], CHLOC[b] = ci_, j * RB
                    b += 1
            xt = {}

            def x_chunk_dma(img, ch, eng):
                r0, nb = CHUNKS[ch]
                rows = min(nb * RB + 2, HP - r0)
                t = xpool.tile([CI, rows, WP], bf16, tag=f"x{img}_{ch}")
                eng.dma_start(out=t, in_=x_d[:, img, r0:r0 + rows, :])
                xt[img, ch] = t

            # Critical path: the first matmul group needs x0 chunk0 plus all
            # 9 taps of w[cot0]; spread those over both HWDGE queues.  All
            # input DMAs are issued up front so they sit ahead of every
            # output store in each queue's ring.
            nc.scalar.dma_start(out=wt[0][:, 0:5, :], in_=w_d[:, 0, 0:5, :])
            x_chunk_dma(0, 0, nc.sync)
            nc.sync.dma_start(out=wt[0][:, 5:9, :], in_=w_d[:, 0, 5:9, :])
            x_chunk_dma(0, 1, nc.scalar)
            nc.sync.dma_start(out=wt[1], in_=w_d[:, 1, :, :])
            x_chunk_dma(0, 2, nc.scalar)
            x_chunk_dma(0, 3, nc.sync)
            nc.scalar.dma_start(out=bt, in_=b_d[:, :])
            for img in range(1, NP_CORE):
                for ch in range(len(CHUNKS)):
                    x_chunk_dma(img, ch, nc.scalar if ch % 2 else nc.sync)

            def conv_block(img, blk, cot, sub=None):
                """One PSUM accumulation group.  sub=None: full 8-row block;
                sub=0/1: 4-row half of the block (short drain tail).

                Taps whose input row is entirely zero padding (kh=0 on the
                top image block, kh=2 on the bottom one) stream one output
                row fewer; they are ordered after a full tap so start=True
                always initializes the whole PSUM tile.
                """
                ch, r0 = CHMAP[blk], CHLOC[blk]
                if sub is None:
                    rb, rr0, orow = RB, r0, blk * RB
                else:
                    rb, rr0, orow = RB // 2, r0 + sub * (RB // 2), \
                        blk * RB + sub * (RB // 2)
                top = orow == 0          # output row 0 in this group
                bot = orow + rb == H     # output row H-1 in this group
                ps = ppool.tile([128, rb, W], f32, tag="ps")
                order = list(range(KH * KW))
                if top:  # kh=0 taps (pos 0..2) trimmed; start on pos 3
                    order = [3, 4, 5, 6, 7, 8, 0, 1, 2]
                for i, pos in enumerate(order):
                    kh, kw = divmod(pos, KW)
                    if top and kh == 0:
                        rhs = xt[img, ch][:, rr0 + 1:rr0 + rb, kw:kw + W]
                        out_ap = ps[:, 1:rb, :]
                    elif bot and kh == 2:
                        rhs = xt[img, ch][:, rr0 + 2:rr0 + 1 + rb,
                                          kw:kw + W]
                        out_ap = ps[:, 0:rb - 1, :]
                    else:
                        rhs = xt[img, ch][:, rr0 + kh:rr0 + kh + rb,
                                          kw:kw + W]
                        out_ap = ps
                    nc.tensor.matmul(
                        out_ap, wslice(pos, cot), rhs,
                        start=(i == 0), stop=(i == KH * KW - 1),
                    )
                return ps, orow, rb

            out_parity = 0

            def store_block(img, cot, ps, orow, rb, eng=None):
                nonlocal out_parity
                ot = opool.tile([128, rb, W], f32)
                nc.vector.tensor_scalar_add(ot, ps, bt[:, cot:cot + 1])
                if eng is None:
                    eng = nc.sync if out_parity == 0 else nc.scalar
                    out_parity ^= 1
                eng.dma_start(out=y_d[cot, :, img, orow:orow + rb, :],
                              in_=ot)

            for img in range(NP_CORE):
                # img0 runs cot-major: its 7 cot0 blocks are ~12us of work
                # gated only on w[cot0] + x chunks, hiding the w[cot1] DMA.
                units = ([(cot, blk) for cot in range(COT)
                          for blk in range(NBLK)] if img == 0 else
                         [(cot, blk) for blk in range(NBLK)
                          for cot in range(COT)])
                for cot, blk in units:
                    last = (img == NP_CORE - 1 and cot == COT - 1
                            and blk == NBLK - 1)
                    if last:
                        # Final block as two 4-row groups on both queues so
                        # the second group's matmuls hide the first's drain.
                        ps, orow, rb = conv_block(img, blk, cot, sub=0)
                        store_block(img, cot, ps, orow, rb, eng=nc.sync)
                        ps, orow, rb = conv_block(img, blk, cot, sub=1)
                        store_block(img, cot, ps, orow, rb, eng=nc.scalar)
                    else:
                        ps, orow, rb = conv_block(img, blk, cot)
                        store_block(img, cot, ps, orow, rb)

    nc.compile()
    return nc


def prep_in_maps(input, weight, bias):
    """Host-side layout prep -> one in_map per core."""
    import ml_dtypes

    bf16 = ml_dtypes.bfloat16
    xp = np.pad(input, ((0, 0), (0, 0), (1, 1), (1, 1))).astype(bf16)
    # weight [co, ci, kh, kw] -> [ci, cot, (kh kw), cop]
    wr = np.ascontiguousarray(
        weight.transpose(1, 2, 3, 0).reshape(CI, KH * KW, COT, 128)
        .transpose(0, 2, 1, 3)).astype(bf16)
    b2 = np.ascontiguousarray(bias.reshape(COT, 128).T)

    in_maps = []
    for c in range(N_CORES):
        xc = np.ascontiguousarray(
            xp[c * NP_CORE:(c + 1) * NP_CORE].transpose(1, 0, 2, 3))
        in_maps.append({"x": xc, "w": wr, "b2": b2})
    return in_maps


def kernel(input, weight, bias):
    input = np.asarray(input, dtype=np.float32)
    weight = np.asarray(weight, dtype=np.float32)
    bias = np.asarray(bias, dtype=np.float32)

    if "nc" not in _CACHE:
        _CACHE["nc"] = _build_program()
    nc = _CACHE["nc"]

    from concourse.bass_utils import run_bass_kernel_spmd

    in_maps = prep_in_maps(input, weight, bias)
    res = run_bass_kernel_spmd(nc, in_maps, core_ids=list(range(N_CORES)))

    out = np.empty((N, CO, H, W), dtype=np.float32)
    for c in range(N_CORES):
        y = res.results[c]["y"]  # [COT, 128, NP_CORE, H, W]
        out[c * NP_CORE:(c + 1) * NP_CORE] = (
            y.transpose(2, 0, 1, 3, 4).reshape(NP_CORE, CO, H, W))
    return out
